# revision 12
# baseline (speedup 1.0000x reference)
"""Multi-head causal self-attention (B=2, T=2048, D=1024, H=16, Dh=64) on 8 TRN2 cores.

Sharding: data-parallel over batch (2 groups of 4 cores), tensor-parallel over
heads within a group (4 heads/core). Host sums the 4 partial outputs per batch.

Design (vs the fp32r baseline), driven by the TimelineSim cost model:
  - QKV projection in fp8(e4m3) DoubleRow with host-precomputed hi/lo splits of
    x and Wq/Wk/Wv, 3 compensated passes: 0.75x the fp32r PE cost.
  - PV swapped: stationary = exp-scores chunk [128tk, 128tq] (bf16), moving =
    V [128tk, 64] (bf16) -> y[tq, dh] charged 64 rows/instr; fully-masked
    diagonal chunks skipped. Softmax sums via separate 1-column ones-matmul
    chains (1 cycle each). All (head, tq-chunk) chains accumulate start=False
    into pre-zeroed PSUM (2 banks y + 64B sums), dodging the 2KB pending-zero
    clobber.
  - The attention ii-loop is ACT(exp)-bound, so everything else the PE does
    (QKV chains, output projection, yT transposes) is issued as "filler tasks"
    through a dedicated 1-bank PSUM slot interleaved into the ii loop.
  - exp batched per head-pair: [128, 2, 512-col0] across 2 PSUM banks; PV runs
    one ii behind S^T so the PE never waits on the current exp.
  - y normalized on DVE (reciprocal of sums + tensor_scalar) to bf16, then
    PE-transposed (via identity) to yT bf16 for the output projection.
  - Output written as fp16, summed on host in fp32.
"""
import sys

import numpy as np

for _p in ("/opt/trn_rl_repo", "/root/.axon_site/_ro/trn_rl_repo"):
    if _p not in sys.path:
        try:
            import concourse  # noqa: F401
            break
        except ImportError:
            sys.path.append(_p)

import concourse.bass as bass  # noqa: E402
import concourse.tile as tile  # noqa: E402
from concourse import bacc, mybir  # noqa: E402
from concourse.bass_utils import run_bass_kernel_spmd  # noqa: E402

P = 128
T = 2048
D = 1024
NH = 4          # heads per core
DH = 64
F = NH * DH     # per-core head features (256)
DC = D // P     # 8 contraction chunks
TJ = T // 512   # 4 tq slices
TC = T // P     # 16 tk chunks
N_CORES = 8
FR = mybir.dt.float32r
F32 = mybir.dt.float32
BF16 = mybir.dt.bfloat16
FP16 = mybir.dt.float16
FP8 = mybir.dt.float8e4
AF = mybir.ActivationFunctionType
DRow = mybir.MatmulPerfMode.DoubleRow

NP8 = mybir.dt.np(FP8)
NPBF = mybir.dt.np(BF16)
W_SCALE = 32.0


def build():
    nc = bacc.Bacc("TRN2", target_bir_lowering=False, debug=False, num_devices=N_CORES)
    xh = nc.dram_tensor("xh", [D, T], FP8, kind="ExternalInput").ap()
    xl = nc.dram_tensor("xl", [D, T], FP8, kind="ExternalInput").ap()
    wqkv = {}
    for w in ("q", "k", "v"):
        for s in ("h", "l"):
            nm = f"w{w}{s}"
            # host-packed [P, DC, F]: contiguous 2KB per partition
            wqkv[nm] = nc.dram_tensor(nm, [P, DC * F], FP8, kind="ExternalInput").ap()
    woT = nc.dram_tensor("woT", [P, 2 * D], BF16, kind="ExternalInput").ap()
    mask = nc.dram_tensor("mask", [P, 4 * 512], BF16, kind="ExternalInput").ap()
    idm = nc.dram_tensor("idm", [P, P], BF16, kind="ExternalInput").ap()
    out = nc.dram_tensor("out", [T, D], FP16, kind="ExternalOutput").ap()

    # Wq/Wk/Wv are host-scaled by WS to clear fp8's subnormal range; q,k each
    # carry a factor WS so the logit scale divides by WS^2. v carries WS,
    # cancelled host-side by Wout/WS.
    scale = 1.0 / np.sqrt(DH) / (W_SCALE * W_SCALE)

    with tile.TileContext(nc) as tc:
        with (
            tc.tile_pool(name="weights", bufs=1) as wpool,
            tc.tile_pool(name="persist", bufs=1) as persist,
            tc.tile_pool(name="x", bufs=2) as xpool,
            tc.tile_pool(name="sexp", bufs=6) as sepool,
            tc.tile_pool(name="small", bufs=8) as small,
            tc.tile_pool(name="ysb", bufs=6) as ypool,
            tc.tile_pool(name="outsb", bufs=6) as opool,
            tc.tile_pool(name="ps", bufs=2, space="PSUM") as ps,
            tc.tile_pool(name="ps_y", bufs=1, space="PSUM") as ps_y,
            tc.tile_pool(name="ps_f", bufs=1, space="PSUM") as ps_f,
        ):
            w_sb = {nm: wpool.tile([P, DC, F], FP8, name=nm) for nm in wqkv}
            wo_sb = wpool.tile([P, 2, D], BF16)
            mask_sb = wpool.tile([P, 4, 512], BF16)
            id_sb = wpool.tile([P, P], BF16)
            w_r = {nm: ap.rearrange("p (o f) -> p o f", f=F) for nm, ap in wqkv.items()}
            wo_r = woT.rearrange("p (g e) -> p g e", g=2)
            xh_r = xh.rearrange("(o p) t -> p o t", p=P)
            xl_r = xl.rearrange("(o p) t -> p o t", p=P)

            qT_sb = persist.tile([P, 2, T], FR)
            kT_sb = persist.tile([P, 2, T], FR)
            v_sb = persist.tile([P, NH, TC, DH], BF16)
            yT_sb = persist.tile([P, 2, T], BF16)
            ones_sb = persist.tile([P, 1], BF16)

            # PSUM: S^T ping-pong 2x2 banks (pool ps), y 2 banks + sums 1 bank
            # (pool ps_y), 1 filler bank (pool ps_f).
            ypt = ps_y.tile([P, 2, 512], F32, name="ypt")
            ysum = ps_y.tile([P, 512], F32, name="ysum")
            nc.vector.memset(ypt[:], 0.0)
            nc.vector.memset(ysum[:, 0:16], 0.0)

            x_tiles = {}

            def load_x(j):
                xh_t = xpool.tile([P, DC, 512], FP8, tag="xh", name=f"xh_{j}")
                xl_t = xpool.tile([P, DC, 512], FP8, tag="xl", name=f"xl_{j}")
                nc.sync.dma_start(xh_t[:], xh_r[:, :, 512 * j : 512 * (j + 1)])
                nc.sync.dma_start(xl_t[:], xl_r[:, :, 512 * j : 512 * (j + 1)])
                x_tiles[j] = (xh_t, xl_t)

            xh0 = xpool.tile([P, DC, 512], FP8, tag="xh", name="xh_0")
            xl0 = xpool.tile([P, DC, 512], FP8, tag="xl", name="xl_0")
            x_tiles[0] = (xh0, xl0)
            nc.sync.dma_start(w_sb["wqh"][:], w_r["wqh"][:])
            nc.sync.dma_start(xh0[:], xh_r[:, :, 0:512])
            nc.sync.dma_start(xl0[:], xl_r[:, :, 0:512])
            for nm in ("wql", "wkh", "wkl", "wvh", "wvl"):
                nc.sync.dma_start(w_sb[nm][:], w_r[nm][:])
            nc.sync.dma_start(mask_sb[:], mask.rearrange("p (r q) -> p r q", r=4))
            nc.sync.dma_start(id_sb[:], idm)
            load_x(1)
            nc.sync.dma_start(wo_sb[:], wo_r[:])

            # all-ones column (mask[:, 3, 511] is 1 for every partition)
            nc.vector.tensor_copy(ones_sb[:], mask_sb[:, 3, 511:512])

            def dr_chain(out_ap, w_nm, x_pair, xsl, kind):
                """12 DoubleRow matmuls accumulating hi@hi + hi@lo + lo@hi."""
                xh_t, xl_t = x_pair
                terms = [
                    (w_sb[w_nm + "h"], xh_t),
                    (w_sb[w_nm + "h"], xl_t),
                    (w_sb[w_nm + "l"], xh_t),
                ]
                n = len(terms) * (DC // 2)
                i = 0
                for w_t, x_t in terms:
                    for o2 in range(DC // 2):
                        osl = slice(2 * o2, 2 * o2 + 2)
                        if kind == "qk":
                            lhs = w_t[:, osl, xsl]
                            rhs = x_t[:, osl, :]
                        else:  # v: stationary = x chunk, moving = w
                            lhs = x_t[:, osl, xsl]
                            rhs = w_t[:, osl, :]
                        nc.tensor.matmul(
                            out_ap, lhs, rhs,
                            start=(i == 0), stop=(i == n - 1), perf_mode=DRow,
                        )
                        i += 1

            # ---- filler tasks: PE work threaded through the 1-bank ps_f ----
            def task_qk(j, w_nm, c):
                def run():
                    jsl = slice(512 * j, 512 * (j + 1))
                    dst = qT_sb if w_nm == "q" else kT_sb
                    pt = ps_f.tile([P, 512], F32, tag="f", name=f"{w_nm}_{j}_{c}")
                    dr_chain(pt[:], "w" + w_nm, x_tiles[j],
                             slice(128 * c, 128 * (c + 1)), "qk")
                    nc.vector.tensor_copy(dst[:, c, jsl], pt[:])
                return run

            def task_v(j, i):
                def run():
                    pt = ps_f.tile([P, 512], F32, tag="f", name=f"v_{j}_{i}")
                    dr_chain(pt[:, 0:F], "wv", x_tiles[j],
                             slice(128 * i, 128 * (i + 1)), "v")
                    nc.vector.tensor_copy(
                        v_sb[:, :, 4 * j + i, :],
                        pt[:, 0:F].rearrange("p (h d) -> p h d", h=NH),
                    )
                return run

            def task_tr(j, m, y_bf):
                def run():
                    pt = ps_f.tile([P, 512], F32, tag="f", name=f"tr_{j}_{m}")
                    pt_bf = pt[:].bitcast(BF16)
                    for g in range(2):
                        nc.tensor.transpose(
                            pt_bf[:, 128 * g : 128 * (g + 1)],
                            y_bf[:, 2 * g : 2 * g + 2, :].rearrange("p h d -> p (h d)"),
                            id_sb[:],
                        )
                    nc.vector.tensor_copy(
                        yT_sb[:, :, 512 * j + 128 * m : 512 * j + 128 * (m + 1)],
                        pt_bf[:, 0:256].rearrange("p (g q) -> p g q", g=2),
                    )
                return run

            def task_out(tb, eb):
                def run():
                    pt = ps_f.tile([P, 512], F32, tag="f", name=f"o_{tb}_{eb}")
                    for g in range(2):
                        nc.tensor.matmul(
                            pt[:],
                            yT_sb[:, g, 128 * tb : 128 * (tb + 1)],
                            wo_sb[:, g, 512 * eb : 512 * (eb + 1)],
                            start=(g == 0),
                            stop=(g == 1),
                        )
                    osb = opool.tile([P, 512], FP16, tag="osb", name=f"osb_{tb}_{eb}")
                    nc.vector.tensor_copy(osb[:], pt[:])
                    nc.sync.dma_start(
                        out[128 * tb : 128 * (tb + 1), 512 * eb : 512 * (eb + 1)],
                        osb[:],
                    )
                return run

            def out_wide(tb, use_act):
                # tail-only: both eb chains through a wide ps tile; staging
                # copy on ACT (idle after the last exp) or DVE.
                pt = ps.tile([P, 2, 512], F32, tag="ps", name=f"ow_{tb}")
                for eb in range(2):
                    for g in range(2):
                        nc.tensor.matmul(
                            pt[:, eb, :],
                            yT_sb[:, g, 128 * tb : 128 * (tb + 1)],
                            wo_sb[:, g, 512 * eb : 512 * (eb + 1)],
                            start=(g == 0),
                            stop=(g == 1),
                        )
                osb = opool.tile([P, 2, 512], FP16, tag="osbw", name=f"osbw_{tb}")
                # split staging across ACT and DVE so the tail copy halves
                nc.scalar.copy(osb[:, 0, :], pt[:, 0, :])
                nc.vector.tensor_copy(osb[:, 1, :], pt[:, 1, :])
                nc.sync.dma_start(
                    out[128 * tb : 128 * (tb + 1), :],
                    osb[:].rearrange("p c f -> p (c f)"),
                )

            filler = []

            def drain(n=1):
                for _ in range(n):
                    if filler:
                        filler.pop(0)()

            # slice-0 q/k/v projections run up front through the wide ps pool
            def proj0():
                for w_nm, dst in (("q", qT_sb), ("k", kT_sb)):
                    pt = ps.tile([P, 2, 512], F32, tag="ps", name=f"{w_nm}_0")
                    for c in range(2):
                        dr_chain(pt[:, c, :], "w" + w_nm,
                                 x_tiles[0], slice(128 * c, 128 * (c + 1)), "qk")
                    nc.vector.tensor_copy(dst[:, :, 0:512], pt[:])
                for half in range(2):
                    pt = ps.tile([P, 2, 512], F32, tag="ps", name=f"v_0_{half}")
                    for q in range(2):
                        i = 2 * half + q
                        dr_chain(pt[:, q, 0:F], "wv",
                                 x_tiles[0], slice(128 * i, 128 * (i + 1)), "v")
                        nc.vector.tensor_copy(
                            v_sb[:, :, i, :],
                            pt[:, q, 0:F].rearrange("p (h d) -> p h d", h=NH),
                        )

            proj0()

            # y chain layout: bank = m // 2, col slot = 4 * (m % 2) + h
            def ysl(m, h):
                s = 4 * (m % 2) + h
                return (m // 2, slice(64 * s, 64 * s + 64))

            for j in range(TJ):
                nii = 4 * j + 4

                # enqueue filler for this loop: transposes+outproj of j-1 were
                # queued at the end of slice j-1; here add next projections.
                if j + 1 < TJ:
                    for c in range(2):
                        filler.append(task_qk(j + 1, "q", c))
                    for c in range(2):
                        filler.append(task_qk(j + 1, "k", c))
                    for i in range(4):
                        filler.append(task_v(j + 1, i))

                def issue_pv(ii, c, se):
                    r = ii - 4 * j
                    for t in range(2):
                        h = 2 * c + t
                        for m in range(max(r, 0), 4):
                            b, sl = ysl(m, h)
                            # chain (h, m) gets its last contribution at the
                            # diagonal block ii == 4j + m
                            last = ii == 4 * j + m
                            nc.tensor.matmul(
                                ypt[:, b, sl],
                                se[:, t, 128 * m : 128 * (m + 1)],
                                v_sb[:, h, ii, :],
                                start=False,
                                stop=last,
                                skip_group_check=True,
                            )
                            s = 4 * m + h
                            nc.tensor.matmul(
                                ysum[:, s : s + 1],
                                se[:, t, 128 * m : 128 * (m + 1)],
                                ones_sb[:],
                                start=False,
                                stop=last,
                                skip_group_check=True,
                            )

                def norm_m(m):
                    # chain m fully accumulated: normalize to bf16 and queue
                    # its transpose + output projection as filler.
                    rec4 = small.tile([P, 4], F32, tag="rec", name=f"rec_{j}_{m}")
                    nc.vector.reciprocal(rec4[:], ysum[:, 4 * m : 4 * m + 4])
                    y_bf = ypool.tile([P, NH, DH], BF16, tag="ybf", name=f"ybf_{j}_{m}")
                    for h in range(NH):
                        b, sl = ysl(m, h)
                        nc.vector.tensor_scalar_mul(
                            y_bf[:, h, :], ypt[:, b, sl], rec4[:, h : h + 1]
                        )
                    if j + 1 < TJ and m % 2 == 1:
                        nc.vector.memset(ypt[:, m // 2], 0.0)
                    if j + 1 < TJ:
                        nc.vector.memset(ysum[:, 4 * m : 4 * m + 4], 0.0)
                    return y_bf

                prev = None
                for ii in range(nii):
                    r = ii - 4 * j
                    col0 = min(128 * r, 256) if r > 0 else 0
                    ses = []
                    for c in range(2):
                        sps = ps.tile([P, 2, 512], F32, tag="ps", name=f"s_{j}_{ii}_{c}")
                        for t in range(2):
                            hp = 64 * t
                            nc.tensor.matmul(
                                sps[:, t, col0:],
                                kT_sb[hp : hp + DH, c, 128 * ii : 128 * (ii + 1)],
                                qT_sb[hp : hp + DH, c, 512 * j + col0 : 512 * (j + 1)],
                                start=True,
                                stop=True,
                            )
                        se = sepool.tile([P, 2, 512], BF16, tag="se")
                        nc.scalar.activation(
                            se[:, :, col0:], sps[:, :, col0:], AF.Exp, scale=scale
                        )
                        if r >= 0:
                            msl = slice(128 * r, 128 * (r + 1))
                            nc.gpsimd.tensor_mul(
                                se[:, :, msl],
                                se[:, :, msl],
                                mask_sb[:, r : r + 1, msl].to_broadcast([P, 2, 128]),
                            )
                        ses.append(se)
                        if prev is not None:
                            issue_pv(ii - 1, c, prev[c])
                            if c == 1 and ii - 1 >= 4 * j:
                                # chain m = (ii-1) - 4j is complete
                                m = ii - 1 - 4 * j
                                y_bf = norm_m(m)
                                filler.append(task_tr(j, m, y_bf))
                                filler.append(task_out(4 * j + m, 0))
                                filler.append(task_out(4 * j + m, 1))
                        drain(1)
                    prev = ses

                # flush last PV (chain m=3), then its norm chain.
                issue_pv(nii - 1, 0, prev[0])
                drain(1)
                issue_pv(nii - 1, 1, prev[1])
                y_bf3 = norm_m(3)
                if j + 1 < TJ and j + 2 < TJ:
                    load_x(j + 2)
                if j < TJ - 1:
                    filler.append(task_tr(j, 3, y_bf3))
                    filler.append(task_out(4 * j + 3, 0))
                    filler.append(task_out(4 * j + 3, 1))
                    drain(1)
                else:
                    drain(len(filler))
                    task_tr(j, 3, y_bf3)()
                    out_wide(4 * j + 3, use_act=True)
    nc.compile()
    return nc


def make_mask() -> np.ndarray:
    q = np.arange(512)[None, None, :]
    p = np.arange(P)[:, None, None]
    r = np.arange(4)[None, :, None]
    m = (q >= 128 * r + p).astype(np.float32)
    return np.ascontiguousarray(m.reshape(P, 4 * 512)).astype(NPBF)


def split8(a: np.ndarray):
    hi = a.astype(NP8)
    lo = (a - hi.astype(np.float32)).astype(NP8)
    return hi, lo


def shard_inputs(x, Wqkv, Wout):
    mask = make_mask()
    idm = np.eye(P, dtype=np.float32).astype(NPBF)
    in_maps = []
    for c in range(N_CORES):
        b, g = c // 4, c % 4
        sl = slice(F * g, F * (g + 1))
        xhx, xlx = split8(np.ascontiguousarray(x[b].T))
        m = {"xh": xhx, "xl": xlx, "mask": mask, "idm": idm}
        for w, base in (("q", 0), ("k", D), ("v", 2 * D)):
            # [D, F] -> host-packed [P, DC*F] matching "p (o f)" with D=(o p)
            wT = Wqkv[base:][sl, :].T.reshape(DC, P, F).transpose(1, 0, 2)
            wh, wl = split8(np.ascontiguousarray(wT.reshape(P, DC * F)) * W_SCALE)
            m[f"w{w}h"], m[f"w{w}l"] = wh, wl
        # [F, D] -> [P, 2*D] matching "p (g e)" with F=(g p); /W_SCALE cancels
        # the v-path weight scaling.
        woT = Wout[:, sl].T.reshape(2, P, D).transpose(1, 0, 2) / W_SCALE
        m["woT"] = np.ascontiguousarray(woT.reshape(P, 2 * D)).astype(NPBF)
        in_maps.append(m)
    return in_maps


_NC_CACHE = None


def kernel(x, Wqkv, Wout):
    global _NC_CACHE
    x = np.asarray(x, dtype=np.float32)
    Wqkv = np.asarray(Wqkv, dtype=np.float32)
    Wout = np.asarray(Wout, dtype=np.float32)
    if _NC_CACHE is None:
        _NC_CACHE = build()
    nc = _NC_CACHE
    in_maps = shard_inputs(x, Wqkv, Wout)
    res = run_bass_kernel_spmd(nc, in_maps, core_ids=list(range(N_CORES)))
    outs = [res.results[c]["out"].astype(np.float32) for c in range(N_CORES)]
    return np.stack(
        [outs[0] + outs[1] + outs[2] + outs[3], outs[4] + outs[5] + outs[6] + outs[7]]
    )


# revision 13
# speedup vs baseline: 1.0136x; 1.0136x over previous
"""Multi-head causal self-attention (B=2, T=2048, D=1024, H=16, Dh=64) on 8 TRN2 cores.

Sharding: data-parallel over batch (2 groups of 4 cores), tensor-parallel over
heads within a group (4 heads/core). Host sums the 4 partial outputs per batch.

Design (vs the fp32r baseline), driven by the TimelineSim cost model:
  - QKV projection in fp8(e4m3) DoubleRow with host-precomputed hi/lo splits of
    x and Wq/Wk/Wv, 3 compensated passes: 0.75x the fp32r PE cost.
  - PV swapped: stationary = exp-scores chunk [128tk, 128tq] (bf16), moving =
    V [128tk, 64] (bf16) -> y[tq, dh] charged 64 rows/instr; fully-masked
    diagonal chunks skipped. Softmax sums via separate 1-column ones-matmul
    chains (1 cycle each). All (head, tq-chunk) chains accumulate start=False
    into pre-zeroed PSUM (2 banks y + 64B sums), dodging the 2KB pending-zero
    clobber.
  - The attention ii-loop is ACT(exp)-bound, so everything else the PE does
    (QKV chains, output projection, yT transposes) is issued as "filler tasks"
    through a dedicated 1-bank PSUM slot interleaved into the ii loop.
  - exp batched per head-pair: [128, 2, 512-col0] across 2 PSUM banks; PV runs
    one ii behind S^T so the PE never waits on the current exp.
  - y normalized on DVE (reciprocal of sums + tensor_scalar) to bf16, then
    PE-transposed (via identity) to yT bf16 for the output projection.
  - Output written as fp16, summed on host in fp32.
"""
import sys

import numpy as np

for _p in ("/opt/trn_rl_repo", "/root/.axon_site/_ro/trn_rl_repo"):
    if _p not in sys.path:
        try:
            import concourse  # noqa: F401
            break
        except ImportError:
            sys.path.append(_p)

import concourse.bass as bass  # noqa: E402
import concourse.tile as tile  # noqa: E402
from concourse import bacc, mybir  # noqa: E402
from concourse.bass_utils import run_bass_kernel_spmd  # noqa: E402

P = 128
T = 2048
D = 1024
NH = 4          # heads per core
DH = 64
F = NH * DH     # per-core head features (256)
DC = D // P     # 8 contraction chunks
TJ = T // 512   # 4 tq slices
TC = T // P     # 16 tk chunks
N_CORES = 8
FR = mybir.dt.float32r
F32 = mybir.dt.float32
BF16 = mybir.dt.bfloat16
FP16 = mybir.dt.float16
FP8 = mybir.dt.float8e4
AF = mybir.ActivationFunctionType
DRow = mybir.MatmulPerfMode.DoubleRow

NP8 = mybir.dt.np(FP8)
NPBF = mybir.dt.np(BF16)
W_SCALE = 32.0


def build():
    nc = bacc.Bacc("TRN2", target_bir_lowering=False, debug=False, num_devices=N_CORES)
    xh = nc.dram_tensor("xh", [D, T], FP8, kind="ExternalInput").ap()
    xl = nc.dram_tensor("xl", [D, T], FP8, kind="ExternalInput").ap()
    wqkv = {}
    for w in ("q", "k", "v"):
        for s in ("h", "l"):
            nm = f"w{w}{s}"
            # host-packed [P, DC, F]: contiguous 2KB per partition
            wqkv[nm] = nc.dram_tensor(nm, [P, DC * F], FP8, kind="ExternalInput").ap()
    woT = nc.dram_tensor("woT", [P, 2 * D], BF16, kind="ExternalInput").ap()
    mask = nc.dram_tensor("mask", [P, 4 * 512], BF16, kind="ExternalInput").ap()
    idm = nc.dram_tensor("idm", [P, P], BF16, kind="ExternalInput").ap()
    out = nc.dram_tensor("out", [T, D], FP16, kind="ExternalOutput").ap()

    # Wq/Wk/Wv are host-scaled by WS to clear fp8's subnormal range; q,k each
    # carry a factor WS so the logit scale divides by WS^2. v carries WS,
    # cancelled host-side by Wout/WS.
    scale = 1.0 / np.sqrt(DH) / (W_SCALE * W_SCALE)

    with tile.TileContext(nc) as tc:
        with (
            tc.tile_pool(name="weights", bufs=1) as wpool,
            tc.tile_pool(name="persist", bufs=1) as persist,
            tc.tile_pool(name="x", bufs=2) as xpool,
            tc.tile_pool(name="sexp", bufs=6) as sepool,
            tc.tile_pool(name="small", bufs=8) as small,
            tc.tile_pool(name="ysb", bufs=6) as ypool,
            tc.tile_pool(name="outsb", bufs=6) as opool,
            tc.tile_pool(name="ps", bufs=2, space="PSUM") as ps,
            tc.tile_pool(name="ps_y", bufs=1, space="PSUM") as ps_y,
            tc.tile_pool(name="ps_f", bufs=1, space="PSUM") as ps_f,
        ):
            w_sb = {nm: wpool.tile([P, DC, F], FP8, name=nm) for nm in wqkv}
            wo_sb = wpool.tile([P, 2, D], BF16)
            mask_sb = wpool.tile([P, 4, 512], BF16)
            id_sb = wpool.tile([P, P], BF16)
            w_r = {nm: ap.rearrange("p (o f) -> p o f", f=F) for nm, ap in wqkv.items()}
            wo_r = woT.rearrange("p (g e) -> p g e", g=2)
            xh_r = xh.rearrange("(o p) t -> p o t", p=P)
            xl_r = xl.rearrange("(o p) t -> p o t", p=P)

            qT_sb = persist.tile([P, 2, T], FR)
            kT_sb = persist.tile([P, 2, T], FR)
            v_sb = persist.tile([P, NH, TC, DH], BF16)
            yT_sb = persist.tile([P, 2, T], BF16)
            ones_sb = persist.tile([P, 1], BF16)

            # PSUM: S^T ping-pong 2x2 banks (pool ps), y 2 banks + sums 1 bank
            # (pool ps_y), 1 filler bank (pool ps_f).
            ypt = ps_y.tile([P, 2, 512], F32, name="ypt")
            ysum = ps_y.tile([P, 512], F32, name="ysum")
            nc.vector.memset(ypt[:], 0.0)
            nc.vector.memset(ysum[:, 0:16], 0.0)

            x_tiles = {}

            def load_x(j):
                xh_t = xpool.tile([P, DC, 512], FP8, tag="xh", name=f"xh_{j}")
                xl_t = xpool.tile([P, DC, 512], FP8, tag="xl", name=f"xl_{j}")
                nc.sync.dma_start(xh_t[:], xh_r[:, :, 512 * j : 512 * (j + 1)])
                nc.sync.dma_start(xl_t[:], xl_r[:, :, 512 * j : 512 * (j + 1)])
                x_tiles[j] = (xh_t, xl_t)

            xh0 = xpool.tile([P, DC, 512], FP8, tag="xh", name="xh_0")
            xl0 = xpool.tile([P, DC, 512], FP8, tag="xl", name="xl_0")
            x_tiles[0] = (xh0, xl0)
            nc.sync.dma_start(w_sb["wqh"][:], w_r["wqh"][:])
            nc.sync.dma_start(xh0[:], xh_r[:, :, 0:512])
            nc.sync.dma_start(xl0[:], xl_r[:, :, 0:512])
            for nm in ("wkh", "wql", "wkl"):
                nc.sync.dma_start(w_sb[nm][:], w_r[nm][:])
            nc.sync.dma_start(mask_sb[:], mask.rearrange("p (r q) -> p r q", r=4))
            for nm in ("wvh", "wvl"):
                nc.sync.dma_start(w_sb[nm][:], w_r[nm][:])
            nc.sync.dma_start(id_sb[:], idm)
            load_x(1)
            nc.sync.dma_start(wo_sb[:], wo_r[:])

            # all-ones column (mask[:, 3, 511] is 1 for every partition)
            nc.vector.tensor_copy(ones_sb[:], mask_sb[:, 3, 511:512])

            def dr_chain(out_ap, w_nm, x_pair, xsl, kind):
                """12 DoubleRow matmuls accumulating hi@hi + hi@lo + lo@hi."""
                xh_t, xl_t = x_pair
                terms = [
                    (w_sb[w_nm + "h"], xh_t),
                    (w_sb[w_nm + "h"], xl_t),
                    (w_sb[w_nm + "l"], xh_t),
                ]
                n = len(terms) * (DC // 2)
                i = 0
                for w_t, x_t in terms:
                    for o2 in range(DC // 2):
                        osl = slice(2 * o2, 2 * o2 + 2)
                        if kind == "qk":
                            lhs = w_t[:, osl, xsl]
                            rhs = x_t[:, osl, :]
                        else:  # v: stationary = x chunk, moving = w
                            lhs = x_t[:, osl, xsl]
                            rhs = w_t[:, osl, :]
                        nc.tensor.matmul(
                            out_ap, lhs, rhs,
                            start=(i == 0), stop=(i == n - 1), perf_mode=DRow,
                        )
                        i += 1

            # ---- filler tasks: PE work threaded through the 1-bank ps_f ----
            def task_qk(j, w_nm, c):
                def run():
                    jsl = slice(512 * j, 512 * (j + 1))
                    dst = qT_sb if w_nm == "q" else kT_sb
                    pt = ps_f.tile([P, 512], F32, tag="f", name=f"{w_nm}_{j}_{c}")
                    dr_chain(pt[:], "w" + w_nm, x_tiles[j],
                             slice(128 * c, 128 * (c + 1)), "qk")
                    nc.vector.tensor_copy(dst[:, c, jsl], pt[:])
                return run

            def task_v(j, i):
                def run():
                    pt = ps_f.tile([P, 512], F32, tag="f", name=f"v_{j}_{i}")
                    dr_chain(pt[:, 0:F], "wv", x_tiles[j],
                             slice(128 * i, 128 * (i + 1)), "v")
                    nc.vector.tensor_copy(
                        v_sb[:, :, 4 * j + i, :],
                        pt[:, 0:F].rearrange("p (h d) -> p h d", h=NH),
                    )
                run.is_v = True
                return run

            def task_tr(j, m, y_bf):
                def run():
                    pt = ps_f.tile([P, 512], F32, tag="f", name=f"tr_{j}_{m}")
                    pt_bf = pt[:].bitcast(BF16)
                    for g in range(2):
                        nc.tensor.transpose(
                            pt_bf[:, 128 * g : 128 * (g + 1)],
                            y_bf[:, 2 * g : 2 * g + 2, :].rearrange("p h d -> p (h d)"),
                            id_sb[:],
                        )
                    nc.vector.tensor_copy(
                        yT_sb[:, :, 512 * j + 128 * m : 512 * j + 128 * (m + 1)],
                        pt_bf[:, 0:256].rearrange("p (g q) -> p g q", g=2),
                    )
                return run

            def task_out(tb, eb):
                def run():
                    pt = ps_f.tile([P, 512], F32, tag="f", name=f"o_{tb}_{eb}")
                    for g in range(2):
                        nc.tensor.matmul(
                            pt[:],
                            yT_sb[:, g, 128 * tb : 128 * (tb + 1)],
                            wo_sb[:, g, 512 * eb : 512 * (eb + 1)],
                            start=(g == 0),
                            stop=(g == 1),
                        )
                    osb = opool.tile([P, 512], FP16, tag="osb", name=f"osb_{tb}_{eb}")
                    nc.vector.tensor_copy(osb[:], pt[:])
                    nc.sync.dma_start(
                        out[128 * tb : 128 * (tb + 1), 512 * eb : 512 * (eb + 1)],
                        osb[:],
                    )
                return run

            def out_wide(tb, use_act):
                # tail-only: both eb chains through a wide ps tile; staging
                # copy on ACT (idle after the last exp) or DVE.
                pt = ps.tile([P, 2, 512], F32, tag="ps", name=f"ow_{tb}")
                for eb in range(2):
                    for g in range(2):
                        nc.tensor.matmul(
                            pt[:, eb, :],
                            yT_sb[:, g, 128 * tb : 128 * (tb + 1)],
                            wo_sb[:, g, 512 * eb : 512 * (eb + 1)],
                            start=(g == 0),
                            stop=(g == 1),
                        )
                osb = opool.tile([P, 2, 512], FP16, tag="osbw", name=f"osbw_{tb}")
                # split staging across ACT and DVE so the tail copy halves
                nc.scalar.copy(osb[:, 0, :], pt[:, 0, :])
                nc.vector.tensor_copy(osb[:, 1, :], pt[:, 1, :])
                nc.sync.dma_start(
                    out[128 * tb : 128 * (tb + 1), :],
                    osb[:].rearrange("p c f -> p (c f)"),
                )

            filler = []

            def drain(n=1):
                for _ in range(n):
                    if filler:
                        filler.pop(0)()

            # slice-0 q/k/v projections run up front through the wide ps pool
            def proj0():
                # c-major: q+k for chunk c land together so the first S^T of
                # loop 0 can start before chunk c=1 is projected. v(0) runs
                # as loop-0 filler so it doesn't delay the first exp.
                for c in range(2):
                    pt = ps.tile([P, 2, 512], F32, tag="ps", name=f"qk0_{c}")
                    csl = slice(128 * c, 128 * (c + 1))
                    dr_chain(pt[:, 0, :], "wq", x_tiles[0], csl, "qk")
                    dr_chain(pt[:, 1, :], "wk", x_tiles[0], csl, "qk")
                    nc.vector.tensor_copy(qT_sb[:, c, 0:512], pt[:, 0, :])
                    nc.vector.tensor_copy(kT_sb[:, c, 0:512], pt[:, 1, :])

            proj0()
            filler.extend([task_v(0, 0), task_v(0, 1), task_v(0, 2), task_v(0, 3)])

            # y chain layout: bank = m // 2, col slot = 4 * (m % 2) + h
            def ysl(m, h):
                s = 4 * (m % 2) + h
                return (m // 2, slice(64 * s, 64 * s + 64))

            for j in range(TJ):
                nii = 4 * j + 4
                last_ow = []

                # next slice's q/k go near the FRONT of the queue (they gate
                # the next loop's S^T) but behind any pending v tasks of THIS
                # slice, which feed the current loop's PV.
                if j + 1 < TJ:
                    head = [task_qk(j + 1, "q", c) for c in range(2)]
                    head += [task_qk(j + 1, "k", c) for c in range(2)]
                    nv = sum(1 for t in filler if getattr(t, "is_v", False))
                    filler[nv:nv] = head
                    for i in range(4):
                        filler.append(task_v(j + 1, i))

                def issue_pv(ii, c, se):
                    r = ii - 4 * j
                    for t in range(2):
                        h = 2 * c + t
                        for m in range(max(r, 0), 4):
                            b, sl = ysl(m, h)
                            # chain (h, m) gets its last contribution at the
                            # diagonal block ii == 4j + m
                            last = ii == 4 * j + m
                            nc.tensor.matmul(
                                ypt[:, b, sl],
                                se[:, t, 128 * m : 128 * (m + 1)],
                                v_sb[:, h, ii, :],
                                start=False,
                                stop=last,
                                skip_group_check=True,
                            )
                            s = 4 * m + h
                            nc.tensor.matmul(
                                ysum[:, s : s + 1],
                                se[:, t, 128 * m : 128 * (m + 1)],
                                ones_sb[:],
                                start=False,
                                stop=last,
                                skip_group_check=True,
                            )

                def norm_m(m):
                    # chain m fully accumulated: normalize to bf16 and queue
                    # its transpose + output projection as filler.
                    rec4 = small.tile([P, 4], F32, tag="rec", name=f"rec_{j}_{m}")
                    nc.vector.reciprocal(rec4[:], ysum[:, 4 * m : 4 * m + 4])
                    y_bf = ypool.tile([P, NH, DH], BF16, tag="ybf", name=f"ybf_{j}_{m}")
                    for h in range(NH):
                        b, sl = ysl(m, h)
                        nc.vector.tensor_scalar_mul(
                            y_bf[:, h, :], ypt[:, b, sl], rec4[:, h : h + 1]
                        )
                    if j + 1 < TJ and m % 2 == 1:
                        nc.vector.memset(ypt[:, m // 2], 0.0)
                    if j + 1 < TJ:
                        nc.vector.memset(ysum[:, 4 * m : 4 * m + 4], 0.0)
                    return y_bf

                prev = None
                for ii in range(nii):
                    r = ii - 4 * j
                    col0 = min(128 * r, 256) if r > 0 else 0
                    ses = []
                    for c in range(2):
                        sps = ps.tile([P, 2, 512], F32, tag="ps", name=f"s_{j}_{ii}_{c}")
                        for t in range(2):
                            hp = 64 * t
                            nc.tensor.matmul(
                                sps[:, t, col0:],
                                kT_sb[hp : hp + DH, c, 128 * ii : 128 * (ii + 1)],
                                qT_sb[hp : hp + DH, c, 512 * j + col0 : 512 * (j + 1)],
                                start=True,
                                stop=True,
                            )
                        se = sepool.tile([P, 2, 512], BF16, tag="se")
                        nc.scalar.activation(
                            se[:, :, col0:], sps[:, :, col0:], AF.Exp, scale=scale
                        )
                        if r >= 0:
                            msl = slice(128 * r, 128 * (r + 1))
                            nc.gpsimd.tensor_mul(
                                se[:, :, msl],
                                se[:, :, msl],
                                mask_sb[:, r : r + 1, msl].to_broadcast([P, 2, 128]),
                            )
                        ses.append(se)
                        if prev is not None:
                            issue_pv(ii - 1, c, prev[c])
                            if c == 1 and ii - 1 >= 4 * j:
                                # chain m = (ii-1) - 4j is complete
                                m = ii - 1 - 4 * j
                                y_bf = norm_m(m)
                                filler.append(task_tr(j, m, y_bf))
                                if j < TJ - 1:
                                    filler.append(task_out(4 * j + m, 0))
                                    filler.append(task_out(4 * j + m, 1))
                                else:
                                    # last slice: wide 2-bank outproj with
                                    # ACT/DVE-split staging, right away
                                    last_ow.append(4 * j + m)
                        drain(1)
                    prev = ses

                # flush last PV (chain m=3), then its norm chain.
                issue_pv(nii - 1, 0, prev[0])
                drain(1)
                issue_pv(nii - 1, 1, prev[1])
                y_bf3 = norm_m(3)
                if j + 1 < TJ and j + 2 < TJ:
                    load_x(j + 2)
                if j < TJ - 1:
                    filler.append(task_tr(j, 3, y_bf3))
                    filler.append(task_out(4 * j + 3, 0))
                    filler.append(task_out(4 * j + 3, 1))
                    drain(1)
                else:
                    drain(len(filler))
                    for tb in last_ow:
                        out_wide(tb, use_act=True)
                    task_tr(j, 3, y_bf3)()
                    out_wide(4 * j + 3, use_act=True)
    nc.compile()
    return nc


def make_mask() -> np.ndarray:
    q = np.arange(512)[None, None, :]
    p = np.arange(P)[:, None, None]
    r = np.arange(4)[None, :, None]
    m = (q >= 128 * r + p).astype(np.float32)
    return np.ascontiguousarray(m.reshape(P, 4 * 512)).astype(NPBF)


def split8(a: np.ndarray):
    hi = a.astype(NP8)
    lo = (a - hi.astype(np.float32)).astype(NP8)
    return hi, lo


def shard_inputs(x, Wqkv, Wout):
    mask = make_mask()
    idm = np.eye(P, dtype=np.float32).astype(NPBF)
    in_maps = []
    for c in range(N_CORES):
        b, g = c // 4, c % 4
        sl = slice(F * g, F * (g + 1))
        xhx, xlx = split8(np.ascontiguousarray(x[b].T))
        m = {"xh": xhx, "xl": xlx, "mask": mask, "idm": idm}
        for w, base in (("q", 0), ("k", D), ("v", 2 * D)):
            # [D, F] -> host-packed [P, DC*F] matching "p (o f)" with D=(o p)
            wT = Wqkv[base:][sl, :].T.reshape(DC, P, F).transpose(1, 0, 2)
            wh, wl = split8(np.ascontiguousarray(wT.reshape(P, DC * F)) * W_SCALE)
            m[f"w{w}h"], m[f"w{w}l"] = wh, wl
        # [F, D] -> [P, 2*D] matching "p (g e)" with F=(g p); /W_SCALE cancels
        # the v-path weight scaling.
        woT = Wout[:, sl].T.reshape(2, P, D).transpose(1, 0, 2) / W_SCALE
        m["woT"] = np.ascontiguousarray(woT.reshape(P, 2 * D)).astype(NPBF)
        in_maps.append(m)
    return in_maps


_NC_CACHE = None


def kernel(x, Wqkv, Wout):
    global _NC_CACHE
    x = np.asarray(x, dtype=np.float32)
    Wqkv = np.asarray(Wqkv, dtype=np.float32)
    Wout = np.asarray(Wout, dtype=np.float32)
    if _NC_CACHE is None:
        _NC_CACHE = build()
    nc = _NC_CACHE
    in_maps = shard_inputs(x, Wqkv, Wout)
    res = run_bass_kernel_spmd(nc, in_maps, core_ids=list(range(N_CORES)))
    outs = [res.results[c]["out"].astype(np.float32) for c in range(N_CORES)]
    return np.stack(
        [outs[0] + outs[1] + outs[2] + outs[3], outs[4] + outs[5] + outs[6] + outs[7]]
    )


# revision 14
# speedup vs baseline: 1.0155x; 1.0018x over previous
"""Multi-head causal self-attention (B=2, T=2048, D=1024, H=16, Dh=64) on 8 TRN2 cores.

Sharding: data-parallel over batch (2 groups of 4 cores), tensor-parallel over
heads within a group (4 heads/core). Host sums the 4 partial outputs per batch.

Design (vs the fp32r baseline), driven by the TimelineSim cost model:
  - QKV projection in fp8(e4m3) DoubleRow with host-precomputed hi/lo splits of
    x and Wq/Wk/Wv, 3 compensated passes: 0.75x the fp32r PE cost.
  - PV swapped: stationary = exp-scores chunk [128tk, 128tq] (bf16), moving =
    V [128tk, 64] (bf16) -> y[tq, dh] charged 64 rows/instr; fully-masked
    diagonal chunks skipped. Softmax sums via separate 1-column ones-matmul
    chains (1 cycle each). All (head, tq-chunk) chains accumulate start=False
    into pre-zeroed PSUM (2 banks y + 64B sums), dodging the 2KB pending-zero
    clobber.
  - The attention ii-loop is ACT(exp)-bound, so everything else the PE does
    (QKV chains, output projection, yT transposes) is issued as "filler tasks"
    through a dedicated 1-bank PSUM slot interleaved into the ii loop.
  - exp batched per head-pair: [128, 2, 512-col0] across 2 PSUM banks; PV runs
    one ii behind S^T so the PE never waits on the current exp.
  - y normalized on DVE (reciprocal of sums + tensor_scalar) to bf16, then
    PE-transposed (via identity) to yT bf16 for the output projection.
  - Output written as fp16, summed on host in fp32.
"""
import sys

import numpy as np

for _p in ("/opt/trn_rl_repo", "/root/.axon_site/_ro/trn_rl_repo"):
    if _p not in sys.path:
        try:
            import concourse  # noqa: F401
            break
        except ImportError:
            sys.path.append(_p)

import concourse.bass as bass  # noqa: E402
import concourse.tile as tile  # noqa: E402
from concourse import bacc, mybir  # noqa: E402
from concourse.bass_utils import run_bass_kernel_spmd  # noqa: E402

P = 128
T = 2048
D = 1024
NH = 4          # heads per core
DH = 64
F = NH * DH     # per-core head features (256)
DC = D // P     # 8 contraction chunks
TJ = T // 512   # 4 tq slices
TC = T // P     # 16 tk chunks
N_CORES = 8
FR = mybir.dt.float32r
F32 = mybir.dt.float32
BF16 = mybir.dt.bfloat16
FP16 = mybir.dt.float16
FP8 = mybir.dt.float8e4
AF = mybir.ActivationFunctionType
DRow = mybir.MatmulPerfMode.DoubleRow

NP8 = mybir.dt.np(FP8)
NPBF = mybir.dt.np(BF16)
W_SCALE = 32.0


def build():
    nc = bacc.Bacc("TRN2", target_bir_lowering=False, debug=False, num_devices=N_CORES)
    xh = nc.dram_tensor("xh", [D, T], FP8, kind="ExternalInput").ap()
    xl = nc.dram_tensor("xl", [D, T], FP8, kind="ExternalInput").ap()
    wqkv = {}
    for w in ("q", "k", "v"):
        for s in ("h", "l"):
            nm = f"w{w}{s}"
            # host-packed [P, DC, F]: contiguous 2KB per partition
            wqkv[nm] = nc.dram_tensor(nm, [P, DC * F], FP8, kind="ExternalInput").ap()
    woT = nc.dram_tensor("woT", [P, 2 * D], BF16, kind="ExternalInput").ap()
    mask = nc.dram_tensor("mask", [P, 4 * 512], BF16, kind="ExternalInput").ap()
    idm = nc.dram_tensor("idm", [P, P], BF16, kind="ExternalInput").ap()
    out = nc.dram_tensor("out", [T, D], FP16, kind="ExternalOutput").ap()

    # Wq/Wk/Wv are host-scaled by WS to clear fp8's subnormal range; q,k each
    # carry a factor WS so the logit scale divides by WS^2. v carries WS,
    # cancelled host-side by Wout/WS.
    scale = 1.0 / np.sqrt(DH) / (W_SCALE * W_SCALE)

    with tile.TileContext(nc) as tc:
        with (
            tc.tile_pool(name="weights", bufs=1) as wpool,
            tc.tile_pool(name="persist", bufs=1) as persist,
            tc.tile_pool(name="x", bufs=2) as xpool,
            tc.tile_pool(name="sexp", bufs=8) as sepool,
            tc.tile_pool(name="small", bufs=8) as small,
            tc.tile_pool(name="ysb", bufs=6) as ypool,
            tc.tile_pool(name="outsb", bufs=6) as opool,
            tc.tile_pool(name="ps", bufs=2, space="PSUM") as ps,
            tc.tile_pool(name="ps_y", bufs=1, space="PSUM") as ps_y,
            tc.tile_pool(name="ps_f", bufs=1, space="PSUM") as ps_f,
        ):
            w_sb = {nm: wpool.tile([P, DC, F], FP8, name=nm) for nm in wqkv}
            wo_sb = wpool.tile([P, 2, D], BF16)
            mask_sb = wpool.tile([P, 4, 512], BF16)
            id_sb = wpool.tile([P, P], BF16)
            w_r = {nm: ap.rearrange("p (o f) -> p o f", f=F) for nm, ap in wqkv.items()}
            wo_r = woT.rearrange("p (g e) -> p g e", g=2)
            xh_r = xh.rearrange("(o p) t -> p o t", p=P)
            xl_r = xl.rearrange("(o p) t -> p o t", p=P)

            qT_sb = persist.tile([P, 2, T], FR)
            kT_sb = persist.tile([P, 2, T], FR)
            v_sb = persist.tile([P, NH, TC, DH], BF16)
            yT_sb = persist.tile([P, 2, T], BF16)
            ones_sb = persist.tile([P, 1], BF16)

            # PSUM: S^T ping-pong 2x2 banks (pool ps), y 2 banks + sums 1 bank
            # (pool ps_y), 1 filler bank (pool ps_f).
            ypt = ps_y.tile([P, 2, 512], F32, name="ypt")
            ysum = ps_y.tile([P, 512], F32, name="ysum")
            nc.vector.memset(ypt[:], 0.0)
            nc.vector.memset(ysum[:, 0:16], 0.0)

            x_tiles = {}

            def load_x(j):
                xh_t = xpool.tile([P, DC, 512], FP8, tag="xh", name=f"xh_{j}")
                xl_t = xpool.tile([P, DC, 512], FP8, tag="xl", name=f"xl_{j}")
                nc.sync.dma_start(xh_t[:], xh_r[:, :, 512 * j : 512 * (j + 1)])
                nc.sync.dma_start(xl_t[:], xl_r[:, :, 512 * j : 512 * (j + 1)])
                x_tiles[j] = (xh_t, xl_t)

            # PE p-state warmup: ~3us of junk matmuls on a zeroed tile
            # during the initial DMA wait ramps the PE to full clock before
            # the first real projection chain.
            warm = wpool.tile([P, 2, 512], BF16, name="warm")
            nc.gpsimd.memset(warm[:], 0.0)
            wpt = ps_f.tile([P, 512], F32, tag="f", name="warmps")
            for _ in range(16):
                nc.tensor.matmul(
                    wpt[:, 0:128], warm[:, 0, 0:128], warm[:].rearrange("p c f -> p (c f)")[:, 0:128],
                    start=True, stop=True,
                )

            xh0 = xpool.tile([P, DC, 512], FP8, tag="xh", name="xh_0")
            xl0 = xpool.tile([P, DC, 512], FP8, tag="xl", name="xl_0")
            x_tiles[0] = (xh0, xl0)
            nc.sync.dma_start(w_sb["wqh"][:], w_r["wqh"][:])
            nc.sync.dma_start(xh0[:], xh_r[:, :, 0:512])
            nc.sync.dma_start(xl0[:], xl_r[:, :, 0:512])
            for nm in ("wkh", "wql", "wkl"):
                nc.sync.dma_start(w_sb[nm][:], w_r[nm][:])
            nc.sync.dma_start(mask_sb[:], mask.rearrange("p (r q) -> p r q", r=4))
            for nm in ("wvh", "wvl"):
                nc.sync.dma_start(w_sb[nm][:], w_r[nm][:])
            nc.sync.dma_start(id_sb[:], idm)
            load_x(1)
            nc.sync.dma_start(wo_sb[:], wo_r[:])

            # all-ones column (mask[:, 3, 511] is 1 for every partition)
            nc.vector.tensor_copy(ones_sb[:], mask_sb[:, 3, 511:512])

            def dr_chain(out_ap, w_nm, x_pair, xsl, kind):
                """12 DoubleRow matmuls accumulating hi@hi + hi@lo + lo@hi."""
                xh_t, xl_t = x_pair
                terms = [
                    (w_sb[w_nm + "h"], xh_t),
                    (w_sb[w_nm + "h"], xl_t),
                    (w_sb[w_nm + "l"], xh_t),
                ]
                n = len(terms) * (DC // 2)
                i = 0
                for w_t, x_t in terms:
                    for o2 in range(DC // 2):
                        osl = slice(2 * o2, 2 * o2 + 2)
                        if kind == "qk":
                            lhs = w_t[:, osl, xsl]
                            rhs = x_t[:, osl, :]
                        else:  # v: stationary = x chunk, moving = w
                            lhs = x_t[:, osl, xsl]
                            rhs = w_t[:, osl, :]
                        nc.tensor.matmul(
                            out_ap, lhs, rhs,
                            start=(i == 0), stop=(i == n - 1), perf_mode=DRow,
                        )
                        i += 1

            # ---- filler tasks: PE work threaded through the 1-bank ps_f ----
            def task_qk(j, w_nm, c):
                def run():
                    jsl = slice(512 * j, 512 * (j + 1))
                    dst = qT_sb if w_nm == "q" else kT_sb
                    pt = ps_f.tile([P, 512], F32, tag="f", name=f"{w_nm}_{j}_{c}")
                    dr_chain(pt[:], "w" + w_nm, x_tiles[j],
                             slice(128 * c, 128 * (c + 1)), "qk")
                    nc.vector.tensor_copy(dst[:, c, jsl], pt[:])
                return run

            def task_v(j, i):
                def run():
                    pt = ps_f.tile([P, 512], F32, tag="f", name=f"v_{j}_{i}")
                    dr_chain(pt[:, 0:F], "wv", x_tiles[j],
                             slice(128 * i, 128 * (i + 1)), "v")
                    nc.vector.tensor_copy(
                        v_sb[:, :, 4 * j + i, :],
                        pt[:, 0:F].rearrange("p (h d) -> p h d", h=NH),
                    )
                run.is_v = True
                return run

            def task_tr(j, m, y_bf):
                def run():
                    pt = ps_f.tile([P, 512], F32, tag="f", name=f"tr_{j}_{m}")
                    pt_bf = pt[:].bitcast(BF16)
                    for g in range(2):
                        nc.tensor.transpose(
                            pt_bf[:, 128 * g : 128 * (g + 1)],
                            y_bf[:, 2 * g : 2 * g + 2, :].rearrange("p h d -> p (h d)"),
                            id_sb[:],
                        )
                    nc.vector.tensor_copy(
                        yT_sb[:, :, 512 * j + 128 * m : 512 * j + 128 * (m + 1)],
                        pt_bf[:, 0:256].rearrange("p (g q) -> p g q", g=2),
                    )
                return run

            def task_out(tb, eb):
                def run():
                    pt = ps_f.tile([P, 512], F32, tag="f", name=f"o_{tb}_{eb}")
                    for g in range(2):
                        nc.tensor.matmul(
                            pt[:],
                            yT_sb[:, g, 128 * tb : 128 * (tb + 1)],
                            wo_sb[:, g, 512 * eb : 512 * (eb + 1)],
                            start=(g == 0),
                            stop=(g == 1),
                        )
                    osb = opool.tile([P, 512], FP16, tag="osb", name=f"osb_{tb}_{eb}")
                    nc.vector.tensor_copy(osb[:], pt[:])
                    nc.sync.dma_start(
                        out[128 * tb : 128 * (tb + 1), 512 * eb : 512 * (eb + 1)],
                        osb[:],
                    )
                return run

            def out_wide(tb, use_act):
                # tail-only: both eb chains through a wide ps tile; staging
                # copy on ACT (idle after the last exp) or DVE.
                pt = ps.tile([P, 2, 512], F32, tag="ps", name=f"ow_{tb}")
                for eb in range(2):
                    for g in range(2):
                        nc.tensor.matmul(
                            pt[:, eb, :],
                            yT_sb[:, g, 128 * tb : 128 * (tb + 1)],
                            wo_sb[:, g, 512 * eb : 512 * (eb + 1)],
                            start=(g == 0),
                            stop=(g == 1),
                        )
                osb = opool.tile([P, 2, 512], FP16, tag="osbw", name=f"osbw_{tb}")
                # split staging across ACT and DVE so the tail copy halves
                nc.scalar.copy(osb[:, 0, :], pt[:, 0, :])
                nc.vector.tensor_copy(osb[:, 1, :], pt[:, 1, :])
                for eb in range(2):
                    nc.sync.dma_start(
                        out[128 * tb : 128 * (tb + 1), 512 * eb : 512 * (eb + 1)],
                        osb[:, eb, :],
                    )

            filler = []

            def drain(n=1):
                for _ in range(n):
                    if filler:
                        filler.pop(0)()

            # slice-0 q/k/v projections run up front through the wide ps pool
            def proj0():
                # c-major: q+k for chunk c land together so the first S^T of
                # loop 0 can start before chunk c=1 is projected. v(0) runs
                # as loop-0 filler so it doesn't delay the first exp.
                for c in range(2):
                    pt = ps.tile([P, 2, 512], F32, tag="ps", name=f"qk0_{c}")
                    csl = slice(128 * c, 128 * (c + 1))
                    dr_chain(pt[:, 0, :], "wq", x_tiles[0], csl, "qk")
                    dr_chain(pt[:, 1, :], "wk", x_tiles[0], csl, "qk")
                    nc.vector.tensor_copy(qT_sb[:, c, 0:512], pt[:, 0, :])
                    nc.vector.tensor_copy(kT_sb[:, c, 0:512], pt[:, 1, :])

            proj0()
            filler.extend([task_v(0, 0), task_v(0, 1), task_v(0, 2), task_v(0, 3)])

            # y chain layout: bank = m // 2, col slot = 4 * (m % 2) + h
            def ysl(m, h):
                s = 4 * (m % 2) + h
                return (m // 2, slice(64 * s, 64 * s + 64))

            for j in range(TJ):
                nii = 4 * j + 4
                last_ow = []

                # next slice's q/k go near the FRONT of the queue (they gate
                # the next loop's S^T) but behind any pending v tasks of THIS
                # slice, which feed the current loop's PV.
                if j + 1 < TJ:
                    head = [task_qk(j + 1, "q", c) for c in range(2)]
                    head += [task_qk(j + 1, "k", c) for c in range(2)]
                    nv = sum(1 for t in filler if getattr(t, "is_v", False))
                    filler[nv:nv] = head
                    for i in range(4):
                        filler.append(task_v(j + 1, i))

                def issue_pv(ii, c, se):
                    r = ii - 4 * j
                    for t in range(2):
                        h = 2 * c + t
                        for m in range(max(r, 0), 4):
                            b, sl = ysl(m, h)
                            # chain (h, m) gets its last contribution at the
                            # diagonal block ii == 4j + m
                            last = ii == 4 * j + m
                            nc.tensor.matmul(
                                ypt[:, b, sl],
                                se[:, t, 128 * m : 128 * (m + 1)],
                                v_sb[:, h, ii, :],
                                start=False,
                                stop=last,
                                skip_group_check=True,
                            )
                            s = 4 * m + h
                            nc.tensor.matmul(
                                ysum[:, s : s + 1],
                                se[:, t, 128 * m : 128 * (m + 1)],
                                ones_sb[:],
                                start=False,
                                stop=last,
                                skip_group_check=True,
                            )

                def norm_m(m):
                    # chain m fully accumulated: normalize to bf16 and queue
                    # its transpose + output projection as filler.
                    rec4 = small.tile([P, 4], F32, tag="rec", name=f"rec_{j}_{m}")
                    nc.vector.reciprocal(rec4[:], ysum[:, 4 * m : 4 * m + 4])
                    y_bf = ypool.tile([P, NH, DH], BF16, tag="ybf", name=f"ybf_{j}_{m}")
                    for h in range(NH):
                        b, sl = ysl(m, h)
                        nc.vector.tensor_scalar_mul(
                            y_bf[:, h, :], ypt[:, b, sl], rec4[:, h : h + 1]
                        )
                    if j + 1 < TJ and m % 2 == 1:
                        nc.vector.memset(ypt[:, m // 2], 0.0)
                    if j + 1 < TJ:
                        nc.vector.memset(ysum[:, 4 * m : 4 * m + 4], 0.0)
                    return y_bf

                prev = None
                for ii in range(nii):
                    r = ii - 4 * j
                    col0 = min(128 * r, 256) if r > 0 else 0
                    ses = []
                    for c in range(2):
                        sps = ps.tile([P, 2, 512], F32, tag="ps", name=f"s_{j}_{ii}_{c}")
                        for t in range(2):
                            hp = 64 * t
                            nc.tensor.matmul(
                                sps[:, t, col0:],
                                kT_sb[hp : hp + DH, c, 128 * ii : 128 * (ii + 1)],
                                qT_sb[hp : hp + DH, c, 512 * j + col0 : 512 * (j + 1)],
                                start=True,
                                stop=True,
                            )
                        se = sepool.tile([P, 2, 512], BF16, tag="se")
                        nc.scalar.activation(
                            se[:, :, col0:], sps[:, :, col0:], AF.Exp, scale=scale
                        )
                        if r >= 0:
                            msl = slice(128 * r, 128 * (r + 1))
                            nc.gpsimd.tensor_mul(
                                se[:, :, msl],
                                se[:, :, msl],
                                mask_sb[:, r : r + 1, msl].to_broadcast([P, 2, 128]),
                            )
                        ses.append(se)
                        if prev is not None:
                            issue_pv(ii - 1, c, prev[c])
                            if c == 1 and ii - 1 >= 4 * j:
                                # chain m = (ii-1) - 4j is complete
                                m = ii - 1 - 4 * j
                                y_bf = norm_m(m)
                                filler.append(task_tr(j, m, y_bf))
                                if j < TJ - 1:
                                    filler.append(task_out(4 * j + m, 0))
                                    filler.append(task_out(4 * j + m, 1))
                                else:
                                    # last slice: wide 2-bank outproj with
                                    # ACT/DVE-split staging, right away
                                    last_ow.append(4 * j + m)
                        drain(1)
                    prev = ses

                # flush last PV (chain m=3), then its norm chain.
                issue_pv(nii - 1, 0, prev[0])
                drain(1)
                issue_pv(nii - 1, 1, prev[1])
                y_bf3 = norm_m(3)
                if j + 1 < TJ and j + 2 < TJ:
                    load_x(j + 2)
                if j < TJ - 1:
                    filler.append(task_tr(j, 3, y_bf3))
                    filler.append(task_out(4 * j + 3, 0))
                    filler.append(task_out(4 * j + 3, 1))
                    drain(1)
                else:
                    drain(len(filler))
                    for tb in last_ow:
                        out_wide(tb, use_act=True)
                    task_tr(j, 3, y_bf3)()
                    out_wide(4 * j + 3, use_act=True)
    nc.compile()
    return nc


def make_mask() -> np.ndarray:
    q = np.arange(512)[None, None, :]
    p = np.arange(P)[:, None, None]
    r = np.arange(4)[None, :, None]
    m = (q >= 128 * r + p).astype(np.float32)
    return np.ascontiguousarray(m.reshape(P, 4 * 512)).astype(NPBF)


def split8(a: np.ndarray):
    hi = a.astype(NP8)
    lo = (a - hi.astype(np.float32)).astype(NP8)
    return hi, lo


def shard_inputs(x, Wqkv, Wout):
    mask = make_mask()
    idm = np.eye(P, dtype=np.float32).astype(NPBF)
    in_maps = []
    for c in range(N_CORES):
        b, g = c // 4, c % 4
        sl = slice(F * g, F * (g + 1))
        xhx, xlx = split8(np.ascontiguousarray(x[b].T))
        m = {"xh": xhx, "xl": xlx, "mask": mask, "idm": idm}
        for w, base in (("q", 0), ("k", D), ("v", 2 * D)):
            # [D, F] -> host-packed [P, DC*F] matching "p (o f)" with D=(o p)
            wT = Wqkv[base:][sl, :].T.reshape(DC, P, F).transpose(1, 0, 2)
            wh, wl = split8(np.ascontiguousarray(wT.reshape(P, DC * F)) * W_SCALE)
            m[f"w{w}h"], m[f"w{w}l"] = wh, wl
        # [F, D] -> [P, 2*D] matching "p (g e)" with F=(g p); /W_SCALE cancels
        # the v-path weight scaling.
        woT = Wout[:, sl].T.reshape(2, P, D).transpose(1, 0, 2) / W_SCALE
        m["woT"] = np.ascontiguousarray(woT.reshape(P, 2 * D)).astype(NPBF)
        in_maps.append(m)
    return in_maps


_NC_CACHE = None


def kernel(x, Wqkv, Wout):
    global _NC_CACHE
    x = np.asarray(x, dtype=np.float32)
    Wqkv = np.asarray(Wqkv, dtype=np.float32)
    Wout = np.asarray(Wout, dtype=np.float32)
    if _NC_CACHE is None:
        _NC_CACHE = build()
    nc = _NC_CACHE
    in_maps = shard_inputs(x, Wqkv, Wout)
    res = run_bass_kernel_spmd(nc, in_maps, core_ids=list(range(N_CORES)))
    outs = [res.results[c]["out"].astype(np.float32) for c in range(N_CORES)]
    return np.stack(
        [outs[0] + outs[1] + outs[2] + outs[3], outs[4] + outs[5] + outs[6] + outs[7]]
    )


# revision 15
# speedup vs baseline: 1.0161x; 1.0006x over previous
"""Multi-head causal self-attention (B=2, T=2048, D=1024, H=16, Dh=64) on 8 TRN2 cores.

Sharding: data-parallel over batch (2 groups of 4 cores), tensor-parallel over
heads within a group (4 heads/core). Host sums the 4 partial outputs per batch.

Design (vs the fp32r baseline), driven by the TimelineSim cost model:
  - QKV projection in fp8(e4m3) DoubleRow with host-precomputed hi/lo splits of
    x and Wq/Wk/Wv, 3 compensated passes: 0.75x the fp32r PE cost.
  - PV swapped: stationary = exp-scores chunk [128tk, 128tq] (bf16), moving =
    V [128tk, 64] (bf16) -> y[tq, dh] charged 64 rows/instr; fully-masked
    diagonal chunks skipped. Softmax sums via separate 1-column ones-matmul
    chains (1 cycle each). All (head, tq-chunk) chains accumulate start=False
    into pre-zeroed PSUM (2 banks y + 64B sums), dodging the 2KB pending-zero
    clobber.
  - The attention ii-loop is ACT(exp)-bound, so everything else the PE does
    (QKV chains, output projection, yT transposes) is issued as "filler tasks"
    through a dedicated 1-bank PSUM slot interleaved into the ii loop.
  - exp batched per head-pair: [128, 2, 512-col0] across 2 PSUM banks; PV runs
    one ii behind S^T so the PE never waits on the current exp.
  - y normalized on DVE (reciprocal of sums + tensor_scalar) to bf16, then
    PE-transposed (via identity) to yT bf16 for the output projection.
  - Output written as fp16, summed on host in fp32.
"""
import sys

import numpy as np

for _p in ("/opt/trn_rl_repo", "/root/.axon_site/_ro/trn_rl_repo"):
    if _p not in sys.path:
        try:
            import concourse  # noqa: F401
            break
        except ImportError:
            sys.path.append(_p)

import concourse.bass as bass  # noqa: E402
import concourse.tile as tile  # noqa: E402
from concourse import bacc, mybir  # noqa: E402
from concourse.bass_utils import run_bass_kernel_spmd  # noqa: E402

P = 128
T = 2048
D = 1024
NH = 4          # heads per core
DH = 64
F = NH * DH     # per-core head features (256)
DC = D // P     # 8 contraction chunks
TJ = T // 512   # 4 tq slices
TC = T // P     # 16 tk chunks
N_CORES = 8
FR = mybir.dt.float32r
F32 = mybir.dt.float32
BF16 = mybir.dt.bfloat16
FP16 = mybir.dt.float16
FP8 = mybir.dt.float8e4
AF = mybir.ActivationFunctionType
DRow = mybir.MatmulPerfMode.DoubleRow

NP8 = mybir.dt.np(FP8)
NPBF = mybir.dt.np(BF16)
W_SCALE = 32.0


def build():
    nc = bacc.Bacc("TRN2", target_bir_lowering=False, debug=False, num_devices=N_CORES)
    xh = nc.dram_tensor("xh", [D, T], FP8, kind="ExternalInput").ap()
    xl = nc.dram_tensor("xl", [D, T], FP8, kind="ExternalInput").ap()
    wqkv = {}
    for w in ("q", "k", "v"):
        for s in ("h", "l"):
            nm = f"w{w}{s}"
            # host-packed [P, DC, F]: contiguous 2KB per partition
            wqkv[nm] = nc.dram_tensor(nm, [P, DC * F], FP8, kind="ExternalInput").ap()
    woT = nc.dram_tensor("woT", [P, 2 * D], BF16, kind="ExternalInput").ap()
    mask = nc.dram_tensor("mask", [P, 4 * 512], BF16, kind="ExternalInput").ap()
    idm = nc.dram_tensor("idm", [P, P], BF16, kind="ExternalInput").ap()
    out = nc.dram_tensor("out", [T, D], FP16, kind="ExternalOutput").ap()

    # Wq/Wk/Wv are host-scaled by WS to clear fp8's subnormal range; q,k each
    # carry a factor WS so the logit scale divides by WS^2. v carries WS,
    # cancelled host-side by Wout/WS.
    scale = 1.0 / np.sqrt(DH) / (W_SCALE * W_SCALE)

    with tile.TileContext(nc) as tc:
        with (
            tc.tile_pool(name="weights", bufs=1) as wpool,
            tc.tile_pool(name="persist", bufs=1) as persist,
            tc.tile_pool(name="x", bufs=2) as xpool,
            tc.tile_pool(name="sexp", bufs=8) as sepool,
            tc.tile_pool(name="small", bufs=8) as small,
            tc.tile_pool(name="ysb", bufs=6) as ypool,
            tc.tile_pool(name="outsb", bufs=6) as opool,
            tc.tile_pool(name="ps", bufs=2, space="PSUM") as ps,
            tc.tile_pool(name="ps_y", bufs=1, space="PSUM") as ps_y,
            tc.tile_pool(name="ps_f", bufs=1, space="PSUM") as ps_f,
        ):
            w_sb = {nm: wpool.tile([P, DC, F], FP8, name=nm) for nm in wqkv}
            wo_sb = wpool.tile([P, 2, D], BF16)
            mask_sb = wpool.tile([P, 4, 512], BF16)
            id_sb = wpool.tile([P, P], BF16)
            w_r = {nm: ap.rearrange("p (o f) -> p o f", f=F) for nm, ap in wqkv.items()}
            wo_r = woT.rearrange("p (g e) -> p g e", g=2)
            xh_r = xh.rearrange("(o p) t -> p o t", p=P)
            xl_r = xl.rearrange("(o p) t -> p o t", p=P)

            qT_sb = persist.tile([P, 2, T], FR)
            kT_sb = persist.tile([P, 2, T], FR)
            v_sb = persist.tile([P, NH, TC, DH], BF16)
            yT_sb = persist.tile([P, 2, T], BF16)
            ones_sb = persist.tile([P, 1], BF16)

            # PSUM: S^T ping-pong 2x2 banks (pool ps), y 2 banks + sums 1 bank
            # (pool ps_y), 1 filler bank (pool ps_f).
            ypt = ps_y.tile([P, 2, 512], F32, name="ypt")
            ysum = ps_y.tile([P, 512], F32, name="ysum")
            nc.vector.memset(ypt[:], 0.0)
            nc.vector.memset(ysum[:, 0:16], 0.0)

            x_tiles = {}

            def load_x(j):
                xh_t = xpool.tile([P, DC, 512], FP8, tag="xh", name=f"xh_{j}")
                xl_t = xpool.tile([P, DC, 512], FP8, tag="xl", name=f"xl_{j}")
                nc.sync.dma_start(xh_t[:], xh_r[:, :, 512 * j : 512 * (j + 1)])
                nc.sync.dma_start(xl_t[:], xl_r[:, :, 512 * j : 512 * (j + 1)])
                x_tiles[j] = (xh_t, xl_t)

            # PE p-state warmup: ~3us of junk matmuls on a zeroed tile
            # during the initial DMA wait ramps the PE to full clock before
            # the first real projection chain.
            warm = wpool.tile([P, 2, 512], BF16, name="warm")
            nc.gpsimd.memset(warm[:], 0.0)
            wpt = ps_f.tile([P, 512], F32, tag="f", name="warmps")
            for _ in range(16):
                nc.tensor.matmul(
                    wpt[:, 0:128], warm[:, 0, 0:128], warm[:].rearrange("p c f -> p (c f)")[:, 0:128],
                    start=True, stop=True,
                )

            xh0 = xpool.tile([P, DC, 512], FP8, tag="xh", name="xh_0")
            xl0 = xpool.tile([P, DC, 512], FP8, tag="xl", name="xl_0")
            x_tiles[0] = (xh0, xl0)
            nc.sync.dma_start(w_sb["wqh"][:], w_r["wqh"][:])
            nc.sync.dma_start(xh0[:], xh_r[:, :, 0:512])
            nc.sync.dma_start(w_sb["wkh"][:], w_r["wkh"][:])
            nc.sync.dma_start(xl0[:], xl_r[:, :, 0:512])
            for nm in ("wql", "wkl"):
                nc.sync.dma_start(w_sb[nm][:], w_r[nm][:])
            nc.sync.dma_start(mask_sb[:], mask.rearrange("p (r q) -> p r q", r=4))
            for nm in ("wvh", "wvl"):
                nc.sync.dma_start(w_sb[nm][:], w_r[nm][:])
            nc.sync.dma_start(id_sb[:], idm)
            load_x(1)
            nc.sync.dma_start(wo_sb[:], wo_r[:])

            # all-ones column (mask[:, 3, 511] is 1 for every partition)
            nc.vector.tensor_copy(ones_sb[:], mask_sb[:, 3, 511:512])

            def dr_chain(out_ap, w_nm, x_pair, xsl, kind):
                """12 DoubleRow matmuls accumulating hi@hi + hi@lo + lo@hi."""
                xh_t, xl_t = x_pair
                terms = [
                    (w_sb[w_nm + "h"], xh_t),
                    (w_sb[w_nm + "h"], xl_t),
                    (w_sb[w_nm + "l"], xh_t),
                ]
                n = len(terms) * (DC // 2)
                i = 0
                for w_t, x_t in terms:
                    for o2 in range(DC // 2):
                        osl = slice(2 * o2, 2 * o2 + 2)
                        if kind == "qk":
                            lhs = w_t[:, osl, xsl]
                            rhs = x_t[:, osl, :]
                        else:  # v: stationary = x chunk, moving = w
                            lhs = x_t[:, osl, xsl]
                            rhs = w_t[:, osl, :]
                        nc.tensor.matmul(
                            out_ap, lhs, rhs,
                            start=(i == 0), stop=(i == n - 1), perf_mode=DRow,
                        )
                        i += 1

            # ---- filler tasks: PE work threaded through the 1-bank ps_f ----
            def task_qk(j, w_nm, c):
                def run():
                    jsl = slice(512 * j, 512 * (j + 1))
                    dst = qT_sb if w_nm == "q" else kT_sb
                    pt = ps_f.tile([P, 512], F32, tag="f", name=f"{w_nm}_{j}_{c}")
                    dr_chain(pt[:], "w" + w_nm, x_tiles[j],
                             slice(128 * c, 128 * (c + 1)), "qk")
                    nc.vector.tensor_copy(dst[:, c, jsl], pt[:])
                return run

            def task_v(j, i):
                def run():
                    pt = ps_f.tile([P, 512], F32, tag="f", name=f"v_{j}_{i}")
                    dr_chain(pt[:, 0:F], "wv", x_tiles[j],
                             slice(128 * i, 128 * (i + 1)), "v")
                    nc.vector.tensor_copy(
                        v_sb[:, :, 4 * j + i, :],
                        pt[:, 0:F].rearrange("p (h d) -> p h d", h=NH),
                    )
                run.is_v = True
                return run

            def task_tr(j, m, y_bf):
                def run():
                    pt = ps_f.tile([P, 512], F32, tag="f", name=f"tr_{j}_{m}")
                    pt_bf = pt[:].bitcast(BF16)
                    for g in range(2):
                        nc.tensor.transpose(
                            pt_bf[:, 128 * g : 128 * (g + 1)],
                            y_bf[:, 2 * g : 2 * g + 2, :].rearrange("p h d -> p (h d)"),
                            id_sb[:],
                        )
                    nc.vector.tensor_copy(
                        yT_sb[:, :, 512 * j + 128 * m : 512 * j + 128 * (m + 1)],
                        pt_bf[:, 0:256].rearrange("p (g q) -> p g q", g=2),
                    )
                return run

            def task_out(tb, eb):
                def run():
                    pt = ps_f.tile([P, 512], F32, tag="f", name=f"o_{tb}_{eb}")
                    for g in range(2):
                        nc.tensor.matmul(
                            pt[:],
                            yT_sb[:, g, 128 * tb : 128 * (tb + 1)],
                            wo_sb[:, g, 512 * eb : 512 * (eb + 1)],
                            start=(g == 0),
                            stop=(g == 1),
                        )
                    osb = opool.tile([P, 512], FP16, tag="osb", name=f"osb_{tb}_{eb}")
                    nc.vector.tensor_copy(osb[:], pt[:])
                    nc.sync.dma_start(
                        out[128 * tb : 128 * (tb + 1), 512 * eb : 512 * (eb + 1)],
                        osb[:],
                    )
                return run

            def out_wide(tb, use_act):
                # tail-only: both eb chains through a wide ps tile; staging
                # copy on ACT (idle after the last exp) or DVE.
                pt = ps.tile([P, 2, 512], F32, tag="ps", name=f"ow_{tb}")
                for eb in range(2):
                    for g in range(2):
                        nc.tensor.matmul(
                            pt[:, eb, :],
                            yT_sb[:, g, 128 * tb : 128 * (tb + 1)],
                            wo_sb[:, g, 512 * eb : 512 * (eb + 1)],
                            start=(g == 0),
                            stop=(g == 1),
                        )
                osb = opool.tile([P, 2, 512], FP16, tag="osbw", name=f"osbw_{tb}")
                # split staging across ACT and DVE so the tail copy halves
                nc.scalar.copy(osb[:, 0, :], pt[:, 0, :])
                nc.vector.tensor_copy(osb[:, 1, :], pt[:, 1, :])
                for eb in range(2):
                    nc.sync.dma_start(
                        out[128 * tb : 128 * (tb + 1), 512 * eb : 512 * (eb + 1)],
                        osb[:, eb, :],
                    )

            filler = []

            def drain(n=1):
                for _ in range(n):
                    if filler:
                        filler.pop(0)()

            # slice-0 q/k/v projections run up front through the wide ps pool
            def proj0():
                # c-major: q+k for chunk c land together so the first S^T of
                # loop 0 can start before chunk c=1 is projected. v(0) runs
                # as loop-0 filler so it doesn't delay the first exp.
                for c in range(2):
                    pt = ps.tile([P, 2, 512], F32, tag="ps", name=f"qk0_{c}")
                    csl = slice(128 * c, 128 * (c + 1))
                    dr_chain(pt[:, 0, :], "wq", x_tiles[0], csl, "qk")
                    dr_chain(pt[:, 1, :], "wk", x_tiles[0], csl, "qk")
                    nc.vector.tensor_copy(qT_sb[:, c, 0:512], pt[:, 0, :])
                    nc.vector.tensor_copy(kT_sb[:, c, 0:512], pt[:, 1, :])

            proj0()
            filler.extend([task_v(0, 0), task_v(0, 1), task_v(0, 2), task_v(0, 3)])

            # y chain layout: bank = m // 2, col slot = 4 * (m % 2) + h
            def ysl(m, h):
                s = 4 * (m % 2) + h
                return (m // 2, slice(64 * s, 64 * s + 64))

            for j in range(TJ):
                nii = 4 * j + 4
                last_ow = []

                # next slice's q/k go near the FRONT of the queue (they gate
                # the next loop's S^T) but behind any pending v tasks of THIS
                # slice, which feed the current loop's PV.
                if j + 1 < TJ:
                    head = [task_qk(j + 1, "q", c) for c in range(2)]
                    head += [task_qk(j + 1, "k", c) for c in range(2)]
                    nv = sum(1 for t in filler if getattr(t, "is_v", False))
                    filler[nv:nv] = head
                    for i in range(4):
                        filler.append(task_v(j + 1, i))

                def issue_pv(ii, c, se):
                    r = ii - 4 * j
                    for t in range(2):
                        h = 2 * c + t
                        for m in range(max(r, 0), 4):
                            b, sl = ysl(m, h)
                            # chain (h, m) gets its last contribution at the
                            # diagonal block ii == 4j + m
                            last = ii == 4 * j + m
                            nc.tensor.matmul(
                                ypt[:, b, sl],
                                se[:, t, 128 * m : 128 * (m + 1)],
                                v_sb[:, h, ii, :],
                                start=False,
                                stop=last,
                                skip_group_check=True,
                            )
                            s = 4 * m + h
                            nc.tensor.matmul(
                                ysum[:, s : s + 1],
                                se[:, t, 128 * m : 128 * (m + 1)],
                                ones_sb[:],
                                start=False,
                                stop=last,
                                skip_group_check=True,
                            )

                def norm_m(m):
                    # chain m fully accumulated: normalize to bf16 and queue
                    # its transpose + output projection as filler.
                    rec4 = small.tile([P, 4], F32, tag="rec", name=f"rec_{j}_{m}")
                    nc.vector.reciprocal(rec4[:], ysum[:, 4 * m : 4 * m + 4])
                    y_bf = ypool.tile([P, NH, DH], BF16, tag="ybf", name=f"ybf_{j}_{m}")
                    for h in range(NH):
                        b, sl = ysl(m, h)
                        nc.vector.tensor_scalar_mul(
                            y_bf[:, h, :], ypt[:, b, sl], rec4[:, h : h + 1]
                        )
                    if j + 1 < TJ and m % 2 == 1:
                        nc.vector.memset(ypt[:, m // 2], 0.0)
                    if j + 1 < TJ:
                        nc.vector.memset(ysum[:, 4 * m : 4 * m + 4], 0.0)
                    return y_bf

                prev = None
                for ii in range(nii):
                    r = ii - 4 * j
                    col0 = min(128 * r, 256) if r > 0 else 0
                    ses = []
                    for c in range(2):
                        sps = ps.tile([P, 2, 512], F32, tag="ps", name=f"s_{j}_{ii}_{c}")
                        for t in range(2):
                            hp = 64 * t
                            nc.tensor.matmul(
                                sps[:, t, col0:],
                                kT_sb[hp : hp + DH, c, 128 * ii : 128 * (ii + 1)],
                                qT_sb[hp : hp + DH, c, 512 * j + col0 : 512 * (j + 1)],
                                start=True,
                                stop=True,
                            )
                        se = sepool.tile([P, 2, 512], BF16, tag="se")
                        nc.scalar.activation(
                            se[:, :, col0:], sps[:, :, col0:], AF.Exp, scale=scale
                        )
                        if r >= 0:
                            msl = slice(128 * r, 128 * (r + 1))
                            nc.gpsimd.tensor_mul(
                                se[:, :, msl],
                                se[:, :, msl],
                                mask_sb[:, r : r + 1, msl].to_broadcast([P, 2, 128]),
                            )
                        ses.append(se)
                        if prev is not None:
                            issue_pv(ii - 1, c, prev[c])
                            if c == 1 and ii - 1 >= 4 * j:
                                # chain m = (ii-1) - 4j is complete
                                m = ii - 1 - 4 * j
                                y_bf = norm_m(m)
                                filler.append(task_tr(j, m, y_bf))
                                if j < TJ - 1:
                                    filler.append(task_out(4 * j + m, 0))
                                    filler.append(task_out(4 * j + m, 1))
                                else:
                                    # last slice: wide 2-bank outproj with
                                    # ACT/DVE-split staging, right away
                                    last_ow.append(4 * j + m)
                        drain(1)
                    prev = ses

                # flush last PV (chain m=3), then its norm chain.
                issue_pv(nii - 1, 0, prev[0])
                drain(1)
                issue_pv(nii - 1, 1, prev[1])
                if j < TJ - 1:
                    y_bf3 = norm_m(3)
                else:
                    # tail: ACT is idle now - split the 4 norm muls ACT/DVE
                    rec4 = small.tile([P, 4], F32, tag="rec", name="rec_last")
                    nc.vector.reciprocal(rec4[:], ysum[:, 12:16])
                    y_bf3 = ypool.tile([P, NH, DH], BF16, tag="ybf", name="ybf_last")
                    for h in range(NH):
                        b, sl = ysl(3, h)
                        if h < 2:
                            nc.scalar.activation(
                                y_bf3[:, h, :], ypt[:, b, sl], AF.Copy,
                                scale=rec4[:, h : h + 1],
                            )
                        else:
                            nc.vector.tensor_scalar_mul(
                                y_bf3[:, h, :], ypt[:, b, sl], rec4[:, h : h + 1]
                            )
                if j + 1 < TJ and j + 2 < TJ:
                    load_x(j + 2)
                if j < TJ - 1:
                    filler.append(task_tr(j, 3, y_bf3))
                    filler.append(task_out(4 * j + 3, 0))
                    filler.append(task_out(4 * j + 3, 1))
                    drain(1)
                else:
                    drain(len(filler))
                    for tb in last_ow:
                        out_wide(tb, use_act=True)
                    task_tr(j, 3, y_bf3)()
                    out_wide(4 * j + 3, use_act=True)
    nc.compile()
    return nc


def make_mask() -> np.ndarray:
    q = np.arange(512)[None, None, :]
    p = np.arange(P)[:, None, None]
    r = np.arange(4)[None, :, None]
    m = (q >= 128 * r + p).astype(np.float32)
    return np.ascontiguousarray(m.reshape(P, 4 * 512)).astype(NPBF)


def split8(a: np.ndarray):
    hi = a.astype(NP8)
    lo = (a - hi.astype(np.float32)).astype(NP8)
    return hi, lo


def shard_inputs(x, Wqkv, Wout):
    mask = make_mask()
    idm = np.eye(P, dtype=np.float32).astype(NPBF)
    in_maps = []
    for c in range(N_CORES):
        b, g = c // 4, c % 4
        sl = slice(F * g, F * (g + 1))
        xhx, xlx = split8(np.ascontiguousarray(x[b].T))
        m = {"xh": xhx, "xl": xlx, "mask": mask, "idm": idm}
        for w, base in (("q", 0), ("k", D), ("v", 2 * D)):
            # [D, F] -> host-packed [P, DC*F] matching "p (o f)" with D=(o p)
            wT = Wqkv[base:][sl, :].T.reshape(DC, P, F).transpose(1, 0, 2)
            wh, wl = split8(np.ascontiguousarray(wT.reshape(P, DC * F)) * W_SCALE)
            m[f"w{w}h"], m[f"w{w}l"] = wh, wl
        # [F, D] -> [P, 2*D] matching "p (g e)" with F=(g p); /W_SCALE cancels
        # the v-path weight scaling.
        woT = Wout[:, sl].T.reshape(2, P, D).transpose(1, 0, 2) / W_SCALE
        m["woT"] = np.ascontiguousarray(woT.reshape(P, 2 * D)).astype(NPBF)
        in_maps.append(m)
    return in_maps


_NC_CACHE = None


def kernel(x, Wqkv, Wout):
    global _NC_CACHE
    x = np.asarray(x, dtype=np.float32)
    Wqkv = np.asarray(Wqkv, dtype=np.float32)
    Wout = np.asarray(Wout, dtype=np.float32)
    if _NC_CACHE is None:
        _NC_CACHE = build()
    nc = _NC_CACHE
    in_maps = shard_inputs(x, Wqkv, Wout)
    res = run_bass_kernel_spmd(nc, in_maps, core_ids=list(range(N_CORES)))
    outs = [res.results[c]["out"].astype(np.float32) for c in range(N_CORES)]
    return np.stack(
        [outs[0] + outs[1] + outs[2] + outs[3], outs[4] + outs[5] + outs[6] + outs[7]]
    )
